# revision 5
# baseline (speedup 1.0000x reference)
"""KNN classifier kernel for Trainium2 (8 NeuronCores, Bass/Tile).

Strategy (classic distributed KNN, train-sharded):
  - Shard X_train/y_train along N_train: 12500 rows per core (padded 12544).
  - Per core: score[q, t] = X[q]·T[t] - 0.5*||T[t]||^2  (monotone in -dist)
    computed as float32r matmuls (full-rate fp32 on the PE) with the
    -0.5*||t||^2 term folded in as an extra K=1 accumulation row.
  - DVE hardware top-8 (InstMax + InstMaxIndex) per 2048-wide superblock,
    then a final top-8 over the 56 superblock candidates per query tile.
  - Each core emits [2048, 8] candidate (value, local index) pairs; the
    host all-gathers the 8*8=64 candidates per query, takes the final
    top-8 and majority-votes the labels (tie -> smallest class).
"""

import numpy as np

import concourse.bass as bass
import concourse.mybir as mybir
import concourse.bacc as bacc
import concourse.tile as tile
import concourse.bass_utils as bass_utils

N_TRAIN, D, N_Q, K, NUM_CLASSES = 100000, 512, 2048, 8, 100
N_CORES = 8
NT_SHARD = N_TRAIN // N_CORES          # 12500
NT_PAD = 12544                          # 24*512 + 256, = 98*128
Q_TILES = N_Q // 128                    # 16
# superblocks: (start, width) covering [0, NT_PAD)
SUPERS = [(i * 2048, 2048) for i in range(6)] + [(12288, 256)]
N_SUP = len(SUPERS)                     # 7
CAND = N_SUP * 8                        # 56 candidates per query per core

F32 = mybir.dt.float32
F32R = mybir.dt.float32r
BF16 = mybir.dt.bfloat16
I32 = mybir.dt.int32
U32 = mybir.dt.uint32
AX = mybir.AxisListType
ALU = mybir.AluOpType

_CACHE = {}


def _build():
    nc = bacc.Bacc("TRN2", target_bir_lowering=False, debug=False)

    lhs = nc.dram_tensor("lhs", [D, N_Q], F32R, kind="ExternalInput")         # X^T
    rhs = nc.dram_tensor("rhs", [D, NT_PAD], F32R, kind="ExternalInput")      # T^T
    t2b = nc.dram_tensor("t2b", [3, NT_PAD], BF16, kind="ExternalInput")      # [h; l; ones]
    oval = nc.dram_tensor("cand_val", [N_Q, CAND], F32, kind="ExternalOutput")
    oidx = nc.dram_tensor("cand_idx", [N_Q, CAND], U32, kind="ExternalOutput")

    with tile.TileContext(nc) as tc:
        with (
            tc.tile_pool(name="lhsp", bufs=1) as lhsp,
            tc.tile_pool(name="rhsp", bufs=2) as rhsp,
            tc.tile_pool(name="scorep", bufs=3) as scorep,
            tc.tile_pool(name="psump", bufs=2, space="PSUM") as psump,
            tc.tile_pool(name="candp", bufs=1) as candp,
            tc.tile_pool(name="smallp", bufs=8) as smallp,
            tc.tile_pool(name="constp", bufs=1) as constp,
        ):
            # --- resident tiles ---
            lhs_sb = []
            for dk in range(4):
                t = lhsp.tile([128, N_Q], F32R, tag=f"lhs{dk}")
                nc.sync.dma_start(t[:], lhs[dk * 128:(dk + 1) * 128, :])
                lhs_sb.append(t)
            ones_sb = constp.tile([1, 128], BF16, tag="ones")
            nc.sync.dma_start(ones_sb[0:1, :], t2b[2:3, 0:128])

            cand_val = candp.tile([128, Q_TILES * CAND], F32, tag="cval")
            cand_idx = candp.tile([128, Q_TILES * CAND], U32, tag="cidx")

            # --- phase 1: scores + per-superblock top-8 ---
            for si, (t0, w) in enumerate(SUPERS):
                rhs_sb = []
                for dk in range(4):
                    t = rhsp.tile([128, w], F32R, tag=f"rhs{dk}")
                    nc.sync.dma_start(t[:, 0:w],
                                      rhs[dk * 128:(dk + 1) * 128, t0:t0 + w])
                    rhs_sb.append(t)
                t2h_sb = rhsp.tile([1, w], BF16, tag="t2h")
                nc.sync.dma_start(t2h_sb[0:1, 0:w], t2b[0:1, t0:t0 + w])
                t2l_sb = rhsp.tile([1, w], BF16, tag="t2l")
                nc.sync.dma_start(t2l_sb[0:1, 0:w], t2b[1:2, t0:t0 + w])

                for qt in range(Q_TILES):
                    scores = scorep.tile([128, w], F32, tag="scores")
                    ps = psump.tile([128, w], F32, tag="ps")
                    for c0 in range(0, w, 512):
                        cw = min(512, w - c0)
                        for dk in range(4):
                            nc.tensor.matmul(
                                ps[:, c0:c0 + cw],
                                lhs_sb[dk][:, qt * 128:(qt + 1) * 128],
                                rhs_sb[dk][:, c0:c0 + cw],
                                start=(dk == 0), stop=False)
                        # -0.5*||t||^2 + 256, as two bf16 rank-1 updates.
                        # Regular-mode bf16 matmuls also keep the PE HAM
                        # activity monitor fed, so the array stays at 2.4GHz
                        # (f32r transpose-mode alone never un-throttles it).
                        nc.tensor.matmul(
                            ps[:, c0:c0 + cw], ones_sb[0:1, :],
                            t2h_sb[0:1, c0:c0 + cw],
                            start=False, stop=False, skip_group_check=True)
                        nc.tensor.matmul(
                            ps[:, c0:c0 + cw], ones_sb[0:1, :],
                            t2l_sb[0:1, c0:c0 + cw],
                            start=False, stop=True, skip_group_check=True)
                    nc.scalar.copy(scores[:, 0:w], ps[:, 0:w])

                    col = qt * CAND + si * 8
                    vslice = cand_val[:, col:col + 8]
                    nc.vector.max(vslice, scores[:, 0:w])
                    nc.vector.max_index(cand_idx[:, col:col + 8], vslice,
                                        scores[:, 0:w])

            # --- phase 2: ship all candidates; host does the final merge ---
            for qt in range(Q_TILES):
                nc.sync.dma_start(oval[qt * 128:(qt + 1) * 128, :],
                                  cand_val[:, qt * CAND:(qt + 1) * CAND])
                nc.sync.dma_start(oidx[qt * 128:(qt + 1) * 128, :],
                                  cand_idx[:, qt * CAND:(qt + 1) * CAND])

    nc.compile()
    return nc


def _prep_inputs(X, X_train):
    X = np.ascontiguousarray(np.asarray(X, dtype=np.float32))
    X_train = np.ascontiguousarray(np.asarray(X_train, dtype=np.float32))
    import ml_dtypes
    bf16 = ml_dtypes.bfloat16
    lhs = np.ascontiguousarray(X.T)                       # [512, 2048]
    in_maps = []
    for c in range(N_CORES):
        shard = X_train[c * NT_SHARD:(c + 1) * NT_SHARD]  # [12500, 512]
        t2 = np.einsum("td,td->t", shard, shard, dtype=np.float32)
        rhsm = np.zeros((D, NT_PAD), dtype=np.float32)
        rhsm[:, 0:NT_SHARD] = shard.T
        t2c = (-0.5 * t2 + 256.0).astype(np.float32)      # centered, order-safe
        h = t2c.astype(bf16)
        l = (t2c - h.astype(np.float32)).astype(bf16)
        t2rows = np.zeros((3, NT_PAD), dtype=bf16)
        t2rows[0, 0:NT_SHARD] = h
        t2rows[1, 0:NT_SHARD] = l
        t2rows[0, NT_SHARD:] = bf16(-1.0e30)
        t2rows[2, :] = bf16(1.0)
        in_maps.append({"lhs": lhs, "rhs": rhsm, "t2b": t2rows})
    return in_maps


def _merge_host(results, y_train):
    y_train = np.asarray(y_train)
    sup_off = np.repeat([s[0] for s in SUPERS], 8)[None, :]            # [1, 56]
    vals = np.concatenate([r["cand_val"] for r in results], axis=1)    # [2048, 448]
    gidx = np.concatenate(
        [r["cand_idx"].astype(np.int64) + sup_off + c * NT_SHARD
         for c, r in enumerate(results)], axis=1)
    order = np.argsort(-vals, axis=1, kind="stable")[:, :K]
    top_idx = np.take_along_axis(gidx, order, axis=1)                  # [2048, 8]
    labels = y_train[top_idx]                                          # [2048, 8]
    counts = np.zeros((N_Q, NUM_CLASSES), dtype=np.int32)
    rows = np.repeat(np.arange(N_Q), K)
    np.add.at(counts, (rows, labels.reshape(-1)), 1)
    return counts.argmax(axis=1).astype(y_train.dtype)


def run(X, X_train, y_train, k, trace=False, **trace_kwargs):
    assert int(k) == K
    if "nc" not in _CACHE:
        _CACHE["nc"] = _build()
    nc = _CACHE["nc"]
    in_maps = _prep_inputs(X, X_train)
    res = bass_utils.run_bass_kernel_spmd(
        nc, in_maps, core_ids=list(range(N_CORES)), trace=trace,
        **trace_kwargs)
    y_pred = _merge_host(res.results, y_train)
    return y_pred, res


def kernel(X, X_train, y_train, k):
    y_pred, _ = run(X, X_train, y_train, k)
    return y_pred


# revision 6
# speedup vs baseline: 1.6644x; 1.6644x over previous
"""KNN classifier kernel for Trainium2 (8 NeuronCores, Bass/Tile).

Strategy (classic distributed KNN, train-sharded):
  - Shard X_train/y_train along N_train: 12500 rows per core (padded 12544).
  - Per core: score[q, t] = X[q]·T[t] - 0.5*||T[t]||^2  (monotone in -dist)
    computed as float32r matmuls (full-rate fp32 on the PE) with the
    -0.5*||t||^2 term folded in as an extra K=1 accumulation row.
  - DVE hardware top-8 (InstMax + InstMaxIndex) per 2048-wide superblock,
    then a final top-8 over the 56 superblock candidates per query tile.
  - Each core emits [2048, 8] candidate (value, local index) pairs; the
    host all-gathers the 8*8=64 candidates per query, takes the final
    top-8 and majority-votes the labels (tie -> smallest class).
"""

import numpy as np

import concourse.bass as bass
import concourse.mybir as mybir
import concourse.bacc as bacc
import concourse.tile as tile
import concourse.bass_utils as bass_utils

N_TRAIN, D, N_Q, K, NUM_CLASSES = 100000, 512, 2048, 8, 100
N_CORES = 8
NT_SHARD = N_TRAIN // N_CORES          # 12500
NT_PAD = 12544                          # 24*512 + 256, = 98*128
Q_TILES = N_Q // 128                    # 16
# superblocks: (start, width) covering [0, NT_PAD)
SUPERS = [(i * 2048, 2048) for i in range(6)] + [(12288, 256)]
N_SUP = len(SUPERS)                     # 7
CAND = N_SUP * 8                        # 56 candidates per query per core

F32 = mybir.dt.float32
F32R = mybir.dt.float32r
BF16 = mybir.dt.bfloat16
I32 = mybir.dt.int32
U32 = mybir.dt.uint32
AX = mybir.AxisListType
ALU = mybir.AluOpType

_CACHE = {}


def _build():
    nc = bacc.Bacc("TRN2", target_bir_lowering=False, debug=False)

    lhs = nc.dram_tensor("lhs", [D, N_Q], F32R, kind="ExternalInput")         # X^T
    rhs = nc.dram_tensor("rhs", [D, NT_PAD], F32R, kind="ExternalInput")      # T^T
    t2r = nc.dram_tensor("t2r", [128, NT_PAD], F32, kind="ExternalInput")     # -||t||^2/2 replicated
    oval = nc.dram_tensor("cand_val", [N_Q, CAND], F32, kind="ExternalOutput")
    oidx = nc.dram_tensor("cand_idx", [N_Q, CAND], U32, kind="ExternalOutput")

    with tile.TileContext(nc) as tc:
        with (
            tc.tile_pool(name="lhsp", bufs=1) as lhsp,
            tc.tile_pool(name="rhsp", bufs=2) as rhsp,
            tc.tile_pool(name="scorep", bufs=3) as scorep,
            tc.tile_pool(name="psump", bufs=2, space="PSUM") as psump,
            tc.tile_pool(name="candp", bufs=1) as candp,
            tc.tile_pool(name="smallp", bufs=8) as smallp,
            tc.tile_pool(name="constp", bufs=1) as constp,
        ):
            # --- resident tiles ---
            lhs_sb = []
            for dk in range(4):
                t = lhsp.tile([128, N_Q], F32R, tag=f"lhs{dk}")
                nc.sync.dma_start(t[:], lhs[dk * 128:(dk + 1) * 128, :])
                lhs_sb.append(t)
            t2rep = constp.tile([128, NT_PAD], F32, tag="t2rep")
            nc.sync.dma_start(t2rep[:], t2r[:, :])

            cand_val = candp.tile([128, Q_TILES * CAND], F32, tag="cval")
            cand_idx = candp.tile([128, Q_TILES * CAND], U32, tag="cidx")

            # --- phase 1: scores + per-superblock top-8 ---
            for si, (t0, w) in enumerate(SUPERS):
                rhs_sb = []
                for dk in range(4):
                    t = rhsp.tile([128, w], F32R, tag=f"rhs{dk}")
                    nc.sync.dma_start(t[:, 0:w],
                                      rhs[dk * 128:(dk + 1) * 128, t0:t0 + w])
                    rhs_sb.append(t)

                for qt in range(Q_TILES):
                    scores = scorep.tile([128, w], F32, tag="scores")
                    ps = psump.tile([128, w], F32, tag="ps")
                    for c0 in range(0, w, 512):
                        cw = min(512, w - c0)
                        for dk in range(4):
                            nc.tensor.matmul(
                                ps[:, c0:c0 + cw],
                                lhs_sb[dk][:, qt * 128:(qt + 1) * 128],
                                rhs_sb[dk][:, c0:c0 + cw],
                                start=(dk == 0), stop=(dk == 3))
                    # fused eviction: scores = psum + (-0.5*||t||^2), one
                    # DVE pass; keeps the rank-1 t2 update off the PE.
                    nc.vector.tensor_tensor(scores[:, 0:w], ps[:, 0:w],
                                            t2rep[:, t0:t0 + w], op=ALU.add)

                    col = qt * CAND + si * 8
                    vslice = cand_val[:, col:col + 8]
                    nc.vector.max(vslice, scores[:, 0:w])
                    nc.vector.max_index(cand_idx[:, col:col + 8], vslice,
                                        scores[:, 0:w])

            # --- phase 2: ship all candidates; host does the final merge ---
            for qt in range(Q_TILES):
                nc.sync.dma_start(oval[qt * 128:(qt + 1) * 128, :],
                                  cand_val[:, qt * CAND:(qt + 1) * CAND])
                nc.sync.dma_start(oidx[qt * 128:(qt + 1) * 128, :],
                                  cand_idx[:, qt * CAND:(qt + 1) * CAND])

    nc.compile()
    return nc


def _prep_inputs(X, X_train):
    X = np.ascontiguousarray(np.asarray(X, dtype=np.float32))
    X_train = np.ascontiguousarray(np.asarray(X_train, dtype=np.float32))
    lhs = np.ascontiguousarray(X.T)                       # [512, 2048]
    in_maps = []
    for c in range(N_CORES):
        shard = X_train[c * NT_SHARD:(c + 1) * NT_SHARD]  # [12500, 512]
        t2 = np.einsum("td,td->t", shard, shard, dtype=np.float32)
        rhsm = np.zeros((D, NT_PAD), dtype=np.float32)
        rhsm[:, 0:NT_SHARD] = shard.T
        t2row = np.full((NT_PAD,), -1.0e30, dtype=np.float32)
        t2row[0:NT_SHARD] = -0.5 * t2
        t2rep = np.ascontiguousarray(
            np.broadcast_to(t2row, (128, NT_PAD)))
        in_maps.append({"lhs": lhs, "rhs": rhsm, "t2r": t2rep})
    return in_maps


def _merge_host(results, y_train):
    y_train = np.asarray(y_train)
    sup_off = np.repeat([s[0] for s in SUPERS], 8)[None, :]            # [1, 56]
    vals = np.concatenate([r["cand_val"] for r in results], axis=1)    # [2048, 448]
    gidx = np.concatenate(
        [r["cand_idx"].astype(np.int64) + sup_off + c * NT_SHARD
         for c, r in enumerate(results)], axis=1)
    order = np.argsort(-vals, axis=1, kind="stable")[:, :K]
    top_idx = np.take_along_axis(gidx, order, axis=1)                  # [2048, 8]
    labels = y_train[top_idx]                                          # [2048, 8]
    counts = np.zeros((N_Q, NUM_CLASSES), dtype=np.int32)
    rows = np.repeat(np.arange(N_Q), K)
    np.add.at(counts, (rows, labels.reshape(-1)), 1)
    return counts.argmax(axis=1).astype(y_train.dtype)


def run(X, X_train, y_train, k, trace=False, **trace_kwargs):
    assert int(k) == K
    if "nc" not in _CACHE:
        _CACHE["nc"] = _build()
    nc = _CACHE["nc"]
    in_maps = _prep_inputs(X, X_train)
    res = bass_utils.run_bass_kernel_spmd(
        nc, in_maps, core_ids=list(range(N_CORES)), trace=trace,
        **trace_kwargs)
    y_pred = _merge_host(res.results, y_train)
    return y_pred, res


def kernel(X, X_train, y_train, k):
    y_pred, _ = run(X, X_train, y_train, k)
    return y_pred


# revision 7
# speedup vs baseline: 1.9114x; 1.1484x over previous
"""KNN classifier kernel for Trainium2 (8 NeuronCores, Bass/Tile).

Strategy (classic distributed KNN, train-sharded):
  - Shard X_train/y_train along N_train: 12500 rows per core (padded 12544).
  - Per core: score[q, t] = X[q]·T[t] - 0.5*||T[t]||^2  (monotone in -dist)
    computed as float32r matmuls (full-rate fp32 on the PE) with the
    -0.5*||t||^2 term folded in as an extra K=1 accumulation row.
  - DVE hardware top-8 (InstMax + InstMaxIndex) per 2048-wide superblock,
    then a final top-8 over the 56 superblock candidates per query tile.
  - Each core emits [2048, 8] candidate (value, local index) pairs; the
    host all-gathers the 8*8=64 candidates per query, takes the final
    top-8 and majority-votes the labels (tie -> smallest class).
"""

import numpy as np

import concourse.bass as bass
import concourse.mybir as mybir
import concourse.bacc as bacc
import concourse.tile as tile
import concourse.bass_utils as bass_utils

N_TRAIN, D, N_Q, K, NUM_CLASSES = 100000, 512, 2048, 8, 100
N_CORES = 8
NT_SHARD = N_TRAIN // N_CORES          # 12500
NT_PAD = 12544                          # 24*512 + 256, = 98*128
Q_TILES = N_Q // 128                    # 16
# superblocks: (start, width) covering [0, NT_PAD)
SUPERS = [(i * 2048, 2048) for i in range(6)] + [(12288, 256)]
N_SUP = len(SUPERS)                     # 7
CAND = N_SUP * 8                        # 56 candidates per query per core

F32 = mybir.dt.float32
F32R = mybir.dt.float32r
BF16 = mybir.dt.bfloat16
I32 = mybir.dt.int32
U32 = mybir.dt.uint32
AX = mybir.AxisListType
ALU = mybir.AluOpType

_CACHE = {}


def _build():
    nc = bacc.Bacc("TRN2", target_bir_lowering=False, debug=False)

    lhs = nc.dram_tensor("lhs", [D, N_Q], F32R, kind="ExternalInput")         # X^T
    rhs = nc.dram_tensor("rhs", [D, NT_PAD], F32R, kind="ExternalInput")      # T^T
    t2r = nc.dram_tensor("t2r", [128, NT_PAD], F32, kind="ExternalInput")     # -||t||^2/2 replicated
    oval = nc.dram_tensor("cand_val", [N_Q, CAND], F32, kind="ExternalOutput")
    oidx = nc.dram_tensor("cand_idx", [N_Q, CAND], U32, kind="ExternalOutput")

    with tile.TileContext(nc) as tc:
        with (
            tc.tile_pool(name="lhsp", bufs=1) as lhsp,
            tc.tile_pool(name="rhsp", bufs=2) as rhsp,
            tc.tile_pool(name="scorep", bufs=3) as scorep,
            tc.tile_pool(name="psump", bufs=2, space="PSUM") as psump,
            tc.tile_pool(name="candp", bufs=1) as candp,
            tc.tile_pool(name="smallp", bufs=8) as smallp,
            tc.tile_pool(name="constp", bufs=1) as constp,
        ):
            # --- resident tiles ---
            lhs_sb = []
            for dk in range(4):
                t = lhsp.tile([128, N_Q], F32R, tag=f"lhs{dk}")
                nc.sync.dma_start(t[:], lhs[dk * 128:(dk + 1) * 128, :])
                lhs_sb.append(t)
            t2rep = constp.tile([128, NT_PAD], F32, tag="t2rep")
            nc.sync.dma_start(t2rep[:], t2r[:, :])

            cand_val = candp.tile([128, Q_TILES * CAND], F32, tag="cval")
            cand_idx = candp.tile([128, Q_TILES * CAND], U32, tag="cidx")

            # --- phase 1: scores + per-superblock top-8 ---
            for si, (t0, w) in enumerate(SUPERS):
                rhs_sb = []
                for dk in range(4):
                    t = rhsp.tile([128, w], F32R, tag=f"rhs{dk}")
                    nc.sync.dma_start(t[:, 0:w],
                                      rhs[dk * 128:(dk + 1) * 128, t0:t0 + w])
                    rhs_sb.append(t)

                for qt in range(Q_TILES):
                    scores = scorep.tile([128, w], F32, tag="scores")
                    ps = psump.tile([128, w], F32, tag="ps")
                    for c0 in range(0, w, 512):
                        cw = min(512, w - c0)
                        for dk in range(4):
                            nc.tensor.matmul(
                                ps[:, c0:c0 + cw],
                                lhs_sb[dk][:, qt * 128:(qt + 1) * 128],
                                rhs_sb[dk][:, c0:c0 + cw],
                                start=(dk == 0), stop=(dk == 3))
                    # eviction on the otherwise-idle ACT, then the t2 add
                    # in-place on the otherwise-idle GPSIMD: DVE keeps only
                    # the MAX8/FIND_INDEX8 passes (its 1-port ops co-run
                    # with GPSIMD on the shared SBUF slot).
                    nc.scalar.copy(scores[:, 0:w], ps[:, 0:w])
                    nc.gpsimd.tensor_tensor(scores[:, 0:w], scores[:, 0:w],
                                            t2rep[:, t0:t0 + w], op=ALU.add)

                    col = qt * CAND + si * 8
                    vslice = cand_val[:, col:col + 8]
                    nc.vector.max(vslice, scores[:, 0:w])
                    nc.vector.max_index(cand_idx[:, col:col + 8], vslice,
                                        scores[:, 0:w])

            # --- phase 2: ship all candidates; host does the final merge ---
            for qt in range(Q_TILES):
                nc.sync.dma_start(oval[qt * 128:(qt + 1) * 128, :],
                                  cand_val[:, qt * CAND:(qt + 1) * CAND])
                nc.sync.dma_start(oidx[qt * 128:(qt + 1) * 128, :],
                                  cand_idx[:, qt * CAND:(qt + 1) * CAND])

    nc.compile()
    return nc


def _prep_inputs(X, X_train):
    X = np.ascontiguousarray(np.asarray(X, dtype=np.float32))
    X_train = np.ascontiguousarray(np.asarray(X_train, dtype=np.float32))
    lhs = np.ascontiguousarray(X.T)                       # [512, 2048]
    in_maps = []
    for c in range(N_CORES):
        shard = X_train[c * NT_SHARD:(c + 1) * NT_SHARD]  # [12500, 512]
        t2 = np.einsum("td,td->t", shard, shard, dtype=np.float32)
        rhsm = np.zeros((D, NT_PAD), dtype=np.float32)
        rhsm[:, 0:NT_SHARD] = shard.T
        t2row = np.full((NT_PAD,), -1.0e30, dtype=np.float32)
        t2row[0:NT_SHARD] = -0.5 * t2
        t2rep = np.ascontiguousarray(
            np.broadcast_to(t2row, (128, NT_PAD)))
        in_maps.append({"lhs": lhs, "rhs": rhsm, "t2r": t2rep})
    return in_maps


def _merge_host(results, y_train):
    y_train = np.asarray(y_train)
    sup_off = np.repeat([s[0] for s in SUPERS], 8)[None, :]            # [1, 56]
    vals = np.concatenate([r["cand_val"] for r in results], axis=1)    # [2048, 448]
    gidx = np.concatenate(
        [r["cand_idx"].astype(np.int64) + sup_off + c * NT_SHARD
         for c, r in enumerate(results)], axis=1)
    order = np.argsort(-vals, axis=1, kind="stable")[:, :K]
    top_idx = np.take_along_axis(gidx, order, axis=1)                  # [2048, 8]
    labels = y_train[top_idx]                                          # [2048, 8]
    counts = np.zeros((N_Q, NUM_CLASSES), dtype=np.int32)
    rows = np.repeat(np.arange(N_Q), K)
    np.add.at(counts, (rows, labels.reshape(-1)), 1)
    return counts.argmax(axis=1).astype(y_train.dtype)


def run(X, X_train, y_train, k, trace=False, **trace_kwargs):
    assert int(k) == K
    if "nc" not in _CACHE:
        _CACHE["nc"] = _build()
    nc = _CACHE["nc"]
    in_maps = _prep_inputs(X, X_train)
    res = bass_utils.run_bass_kernel_spmd(
        nc, in_maps, core_ids=list(range(N_CORES)), trace=trace,
        **trace_kwargs)
    y_pred = _merge_host(res.results, y_train)
    return y_pred, res


def kernel(X, X_train, y_train, k):
    y_pred, _ = run(X, X_train, y_train, k)
    return y_pred


# revision 8
# speedup vs baseline: 1.9935x; 1.0430x over previous
"""KNN classifier kernel for Trainium2 (8 NeuronCores, Bass/Tile).

Strategy (classic distributed KNN, train-sharded):
  - Shard X_train/y_train along N_train: 12500 rows per core (padded 12544).
  - Per core: score[q, t] = X[q]·T[t] - 0.5*||T[t]||^2  (monotone in -dist)
    computed as float32r matmuls (full-rate fp32 on the PE) with the
    -0.5*||t||^2 term folded in as an extra K=1 accumulation row.
  - DVE hardware top-8 (InstMax + InstMaxIndex) per 2048-wide superblock,
    then a final top-8 over the 56 superblock candidates per query tile.
  - Each core emits [2048, 8] candidate (value, local index) pairs; the
    host all-gathers the 8*8=64 candidates per query, takes the final
    top-8 and majority-votes the labels (tie -> smallest class).
"""

import numpy as np

import concourse.bass as bass
import concourse.mybir as mybir
import concourse.bacc as bacc
import concourse.tile as tile
import concourse.bass_utils as bass_utils

N_TRAIN, D, N_Q, K, NUM_CLASSES = 100000, 512, 2048, 8, 100
N_CORES = 8
NT_SHARD = N_TRAIN // N_CORES          # 12500
NT_PAD = 12544                          # 24*512 + 256, = 98*128
Q_TILES = N_Q // 128                    # 16
# superblocks: (start, width) covering [0, NT_PAD)
SUPERS = [(i * 2048, 2048) for i in range(6)] + [(12288, 256)]
N_SUP = len(SUPERS)                     # 7
CAND = N_SUP * 8                        # 56 candidates per query per core

F32 = mybir.dt.float32
F32R = mybir.dt.float32r
BF16 = mybir.dt.bfloat16
I32 = mybir.dt.int32
U32 = mybir.dt.uint32
AX = mybir.AxisListType
ALU = mybir.AluOpType

_CACHE = {}


def _build():
    nc = bacc.Bacc("TRN2", target_bir_lowering=False, debug=False)

    lhs = nc.dram_tensor("lhs", [D, N_Q], F32R, kind="ExternalInput")         # X^T
    rhs = nc.dram_tensor("rhs", [D, NT_PAD], F32R, kind="ExternalInput")      # T^T
    t2r = nc.dram_tensor("t2r", [128, NT_PAD], F32, kind="ExternalInput")     # -||t||^2/2 replicated
    oval = nc.dram_tensor("cand_val", [N_Q, CAND], F32, kind="ExternalOutput")
    oidx = nc.dram_tensor("cand_idx", [N_Q, CAND], U32, kind="ExternalOutput")

    with tile.TileContext(nc) as tc:
        with (
            tc.tile_pool(name="lhsp", bufs=1) as lhsp,
            tc.tile_pool(name="rhsp", bufs=2) as rhsp,
            tc.tile_pool(name="scorep", bufs=3) as scorep,
            tc.tile_pool(name="psump", bufs=2, space="PSUM") as psump,
            tc.tile_pool(name="candp", bufs=1) as candp,
            tc.tile_pool(name="smallp", bufs=8) as smallp,
            tc.tile_pool(name="constp", bufs=1) as constp,
        ):
            # --- resident tiles ---
            lhs_sb = []
            for dk in range(4):
                t = lhsp.tile([128, N_Q], F32R, tag=f"lhs{dk}")
                nc.sync.dma_start(t[:], lhs[dk * 128:(dk + 1) * 128, :])
                lhs_sb.append(t)
            t2rep = constp.tile([128, NT_PAD], F32, tag="t2rep")
            nc.sync.dma_start(t2rep[:], t2r[:, :])

            cand_val = candp.tile([128, Q_TILES * CAND], F32, tag="cval")
            cand_idx = candp.tile([128, Q_TILES * CAND], U32, tag="cidx")

            # --- phase 1: scores + per-superblock top-8 ---
            for si, (t0, w) in enumerate(SUPERS):
                rhs_sb = []
                for dk in range(4):
                    t = rhsp.tile([128, w], F32R, tag=f"rhs{dk}")
                    nc.sync.dma_start(t[:, 0:w],
                                      rhs[dk * 128:(dk + 1) * 128, t0:t0 + w])
                    rhs_sb.append(t)

                for qt in range(Q_TILES):
                    scores = scorep.tile([128, w], F32, tag="scores")
                    ps = psump.tile([128, w], F32, tag="ps")
                    for c0 in range(0, w, 512):
                        cw = min(512, w - c0)
                        for dk in range(4):
                            nc.tensor.matmul(
                                ps[:, c0:c0 + cw],
                                lhs_sb[dk][:, qt * 128:(qt + 1) * 128],
                                rhs_sb[dk][:, c0:c0 + cw],
                                start=(dk == 0), stop=(dk == 3))
                    # eviction on the otherwise-idle ACT, then the t2 add
                    # in-place on the otherwise-idle GPSIMD: DVE keeps only
                    # the MAX8/FIND_INDEX8 passes (its 1-port ops co-run
                    # with GPSIMD on the shared SBUF slot).
                    nc.scalar.copy(scores[:, 0:w], ps[:, 0:w])
                    # balance the t2 add across GPSIMD and DVE (GPSIMD's
                    # 2-input stream is ~2.8x slower; DVE has slack for ~1/5)
                    tt_eng = nc.vector if (si * Q_TILES + qt) % 5 == 0 else nc.gpsimd
                    tt_eng.tensor_tensor(scores[:, 0:w], scores[:, 0:w],
                                         t2rep[:, t0:t0 + w], op=ALU.add)

                    col = qt * CAND + si * 8
                    vslice = cand_val[:, col:col + 8]
                    nc.vector.max(vslice, scores[:, 0:w])
                    nc.vector.max_index(cand_idx[:, col:col + 8], vslice,
                                        scores[:, 0:w])

            # --- phase 2: ship all candidates; host does the final merge ---
            for qt in range(Q_TILES):
                nc.sync.dma_start(oval[qt * 128:(qt + 1) * 128, :],
                                  cand_val[:, qt * CAND:(qt + 1) * CAND])
                nc.sync.dma_start(oidx[qt * 128:(qt + 1) * 128, :],
                                  cand_idx[:, qt * CAND:(qt + 1) * CAND])

    nc.compile()
    return nc


def _prep_inputs(X, X_train):
    X = np.ascontiguousarray(np.asarray(X, dtype=np.float32))
    X_train = np.ascontiguousarray(np.asarray(X_train, dtype=np.float32))
    lhs = np.ascontiguousarray(X.T)                       # [512, 2048]
    in_maps = []
    for c in range(N_CORES):
        shard = X_train[c * NT_SHARD:(c + 1) * NT_SHARD]  # [12500, 512]
        t2 = np.einsum("td,td->t", shard, shard, dtype=np.float32)
        rhsm = np.zeros((D, NT_PAD), dtype=np.float32)
        rhsm[:, 0:NT_SHARD] = shard.T
        t2row = np.full((NT_PAD,), -1.0e30, dtype=np.float32)
        t2row[0:NT_SHARD] = -0.5 * t2
        t2rep = np.ascontiguousarray(
            np.broadcast_to(t2row, (128, NT_PAD)))
        in_maps.append({"lhs": lhs, "rhs": rhsm, "t2r": t2rep})
    return in_maps


def _merge_host(results, y_train):
    y_train = np.asarray(y_train)
    sup_off = np.repeat([s[0] for s in SUPERS], 8)[None, :]            # [1, 56]
    vals = np.concatenate([r["cand_val"] for r in results], axis=1)    # [2048, 448]
    gidx = np.concatenate(
        [r["cand_idx"].astype(np.int64) + sup_off + c * NT_SHARD
         for c, r in enumerate(results)], axis=1)
    order = np.argsort(-vals, axis=1, kind="stable")[:, :K]
    top_idx = np.take_along_axis(gidx, order, axis=1)                  # [2048, 8]
    labels = y_train[top_idx]                                          # [2048, 8]
    counts = np.zeros((N_Q, NUM_CLASSES), dtype=np.int32)
    rows = np.repeat(np.arange(N_Q), K)
    np.add.at(counts, (rows, labels.reshape(-1)), 1)
    return counts.argmax(axis=1).astype(y_train.dtype)


def run(X, X_train, y_train, k, trace=False, **trace_kwargs):
    assert int(k) == K
    if "nc" not in _CACHE:
        _CACHE["nc"] = _build()
    nc = _CACHE["nc"]
    in_maps = _prep_inputs(X, X_train)
    res = bass_utils.run_bass_kernel_spmd(
        nc, in_maps, core_ids=list(range(N_CORES)), trace=trace,
        **trace_kwargs)
    y_pred = _merge_host(res.results, y_train)
    return y_pred, res


def kernel(X, X_train, y_train, k):
    y_pred, _ = run(X, X_train, y_train, k)
    return y_pred


# revision 10
# speedup vs baseline: 2.0240x; 1.0153x over previous
"""KNN classifier kernel for Trainium2 (8 NeuronCores, Bass/Tile).

Strategy (classic distributed KNN, train-sharded):
  - Shard X_train/y_train along N_train: 12500 rows per core (padded 12544).
  - Per core: score[q, t] = X[q]·T[t] - 0.5*||T[t]||^2  (monotone in -dist).
    X·T as float32r matmuls (4x-rate fp32 on the PE); the -0.5*||t||^2
    rank-1 term stays OFF the PE: ACT evicts PSUM->SBUF and the t2 plane
    is added in-place by GPSIMD (4/5) and DVE (1/5), balanced so PE, DVE
    and GPSIMD all run ~90%+ occupied.
  - DVE hardware top-8 (MAX8 + FIND_INDEX8) per 2048-wide superblock.
  - Each core emits [2048, 56] candidate (value, uint32 local pos) pairs;
    the host all-gathers the 8*56 candidates per query, takes the final
    top-8 and majority-votes the labels (tie -> smallest class).
"""

import numpy as np

import concourse.bass as bass
import concourse.mybir as mybir
import concourse.bacc as bacc
import concourse.tile as tile
import concourse.bass_utils as bass_utils

N_TRAIN, D, N_Q, K, NUM_CLASSES = 100000, 512, 2048, 8, 100
N_CORES = 8
NT_SHARD = N_TRAIN // N_CORES          # 12500
NT_PAD = 12544                          # 24*512 + 256, = 98*128
Q_TILES = N_Q // 128                    # 16
# superblocks: (start, width) covering [0, NT_PAD)
SUPERS = [(i * 2048, 2048) for i in range(6)] + [(12288, 256)]
N_SUP = len(SUPERS)                     # 7
CAND = N_SUP * 8                        # 56 candidates per query per core

F32 = mybir.dt.float32
F32R = mybir.dt.float32r
BF16 = mybir.dt.bfloat16
I32 = mybir.dt.int32
U32 = mybir.dt.uint32
AX = mybir.AxisListType
ALU = mybir.AluOpType

_CACHE = {}


def _build():
    nc = bacc.Bacc("TRN2", target_bir_lowering=False, debug=False)

    lhs = nc.dram_tensor("lhs", [D, N_Q], F32R, kind="ExternalInput")         # X^T
    rhs = nc.dram_tensor("rhs", [D, NT_PAD], F32R, kind="ExternalInput")      # T^T
    t2r = nc.dram_tensor("t2r", [128, NT_PAD], F32, kind="ExternalInput")     # -||t||^2/2 replicated
    oval = nc.dram_tensor("cand_val", [N_Q, CAND], F32, kind="ExternalOutput")
    oidx = nc.dram_tensor("cand_idx", [N_Q, CAND], U32, kind="ExternalOutput")

    with tile.TileContext(nc) as tc:
        with (
            tc.tile_pool(name="lhsp", bufs=1) as lhsp,
            tc.tile_pool(name="rhsp", bufs=2) as rhsp,
            tc.tile_pool(name="scorep", bufs=4) as scorep,
            tc.tile_pool(name="psump", bufs=2, space="PSUM") as psump,
            tc.tile_pool(name="candp", bufs=1) as candp,
            tc.tile_pool(name="smallp", bufs=8) as smallp,
            tc.tile_pool(name="constp", bufs=1) as constp,
        ):
            # --- resident tiles ---
            lhs_sb = []
            for dk in range(4):
                t = lhsp.tile([128, N_Q], F32R, tag=f"lhs{dk}")
                nc.sync.dma_start(t[:], lhs[dk * 128:(dk + 1) * 128, :])
                lhs_sb.append(t)
            t2rep = constp.tile([128, NT_PAD], F32, tag="t2rep")
            nc.sync.dma_start(t2rep[:], t2r[:, :])

            cand_val = candp.tile([128, Q_TILES * CAND], F32, tag="cval")
            cand_idx = candp.tile([128, Q_TILES * CAND], U32, tag="cidx")

            # --- phase 1: scores + per-superblock top-8 ---
            for si, (t0, w) in enumerate(SUPERS):
                rhs_sb = []
                for dk in range(4):
                    t = rhsp.tile([128, w], F32R, tag=f"rhs{dk}")
                    nc.sync.dma_start(t[:, 0:w],
                                      rhs[dk * 128:(dk + 1) * 128, t0:t0 + w])
                    rhs_sb.append(t)

                for qt in range(Q_TILES):
                    scores = scorep.tile([128, w], F32, tag="scores")
                    ps = psump.tile([128, w], F32, tag="ps")
                    for c0 in range(0, w, 512):
                        cw = min(512, w - c0)
                        for dk in range(4):
                            nc.tensor.matmul(
                                ps[:, c0:c0 + cw],
                                lhs_sb[dk][:, qt * 128:(qt + 1) * 128],
                                rhs_sb[dk][:, c0:c0 + cw],
                                start=(dk == 0), stop=(dk == 3))
                    # eviction on the otherwise-idle ACT, then the t2 add
                    # in-place on the otherwise-idle GPSIMD: DVE keeps only
                    # the MAX8/FIND_INDEX8 passes (its 1-port ops co-run
                    # with GPSIMD on the shared SBUF slot).
                    nc.scalar.copy(scores[:, 0:w], ps[:, 0:w])
                    # balance the t2 add across GPSIMD and DVE (GPSIMD's
                    # 2-input stream is ~2.8x slower; DVE has slack for ~1/5)
                    tt_eng = nc.vector if (si * Q_TILES + qt) % 5 == 0 else nc.gpsimd
                    tt_eng.tensor_tensor(scores[:, 0:w], scores[:, 0:w],
                                         t2rep[:, t0:t0 + w], op=ALU.add)

                    col = qt * CAND + si * 8
                    vslice = cand_val[:, col:col + 8]
                    nc.vector.max(vslice, scores[:, 0:w])
                    nc.vector.max_index(cand_idx[:, col:col + 8], vslice,
                                        scores[:, 0:w])

            # --- phase 2: ship all candidates; host does the final merge ---
            for qt in range(Q_TILES):
                nc.sync.dma_start(oval[qt * 128:(qt + 1) * 128, :],
                                  cand_val[:, qt * CAND:(qt + 1) * CAND])
                nc.sync.dma_start(oidx[qt * 128:(qt + 1) * 128, :],
                                  cand_idx[:, qt * CAND:(qt + 1) * CAND])

    nc.compile()
    return nc


def _prep_inputs(X, X_train):
    X = np.ascontiguousarray(np.asarray(X, dtype=np.float32))
    X_train = np.ascontiguousarray(np.asarray(X_train, dtype=np.float32))
    lhs = np.ascontiguousarray(X.T)                       # [512, 2048]
    in_maps = []
    for c in range(N_CORES):
        shard = X_train[c * NT_SHARD:(c + 1) * NT_SHARD]  # [12500, 512]
        t2 = np.einsum("td,td->t", shard, shard, dtype=np.float32)
        rhsm = np.zeros((D, NT_PAD), dtype=np.float32)
        rhsm[:, 0:NT_SHARD] = shard.T
        t2row = np.full((NT_PAD,), -1.0e30, dtype=np.float32)
        t2row[0:NT_SHARD] = -0.5 * t2
        t2rep = np.ascontiguousarray(
            np.broadcast_to(t2row, (128, NT_PAD)))
        in_maps.append({"lhs": lhs, "rhs": rhsm, "t2r": t2rep})
    return in_maps


def _merge_host(results, y_train):
    y_train = np.asarray(y_train)
    sup_off = np.repeat([s[0] for s in SUPERS], 8)[None, :]            # [1, 56]
    vals = np.concatenate([r["cand_val"] for r in results], axis=1)    # [2048, 448]
    gidx = np.concatenate(
        [r["cand_idx"].astype(np.int64) + sup_off + c * NT_SHARD
         for c, r in enumerate(results)], axis=1)
    order = np.argsort(-vals, axis=1, kind="stable")[:, :K]
    top_idx = np.take_along_axis(gidx, order, axis=1)                  # [2048, 8]
    labels = y_train[top_idx]                                          # [2048, 8]
    counts = np.zeros((N_Q, NUM_CLASSES), dtype=np.int32)
    rows = np.repeat(np.arange(N_Q), K)
    np.add.at(counts, (rows, labels.reshape(-1)), 1)
    return counts.argmax(axis=1).astype(y_train.dtype)


def run(X, X_train, y_train, k, trace=False, **trace_kwargs):
    assert int(k) == K
    if "nc" not in _CACHE:
        _CACHE["nc"] = _build()
    nc = _CACHE["nc"]
    in_maps = _prep_inputs(X, X_train)
    res = bass_utils.run_bass_kernel_spmd(
        nc, in_maps, core_ids=list(range(N_CORES)), trace=trace,
        **trace_kwargs)
    y_pred = _merge_host(res.results, y_train)
    return y_pred, res


def kernel(X, X_train, y_train, k):
    y_pred, _ = run(X, X_train, y_train, k)
    return y_pred
